# revision 19
# baseline (speedup 1.0000x reference)
"""Trainium2 Bass kernel for nn_MembraneLayer: h = x @ w followed by a
double first-order recurrence over time, producing (syn_rec, mem_rec).

Sharding: data-parallel over batch. 8 cores x 64 batches each.

Radix-2 "weight-folded" design (v5). DVE scans only ODD timesteps and
the evens are recovered algebraically:

  syn[2k+1] = a^2 syn[2k-1] + (a*h[2k-1] + h[2k])   <- scan over g
  a*syn[2k] = syn[2k+1] - h[2k]                     <- TT sub (fp16 2x)
  V[k] := a*v[2k+1] = b^2 V[k-1] + ab*syn[2k-1] + a*syn[2k]  <- scan g_m
  ab*v[2k]  = V[k] - a*syn[2k]                      <- TT sub
  (v = mem/(1-b); host applies the final per-channel scales)

The pair-combines fold into the PE via host-premultiplied weight
copies (w*a for odd-t columns, w for even-t): planes Po = a*h_odd
(leading zero col) and Pe = h_even at baseline matmul cost.

v5 is organized around the measured DMA law: each queue dispatches one
partition-row packet per ~49 ns, so queue throughput = row_bytes/49ns
and a [128, W] DMA costs ~6.3 us of queue time REGARDLESS of W. Hence:
 - x lives in 6 fully-resident [128, 6400] tiles (12800B rows, one per
   ktile; 768 packets total), loaded as [64, .] halves round-robined
   over the sync/gpsimd/scalar/vector queues -> ready in ~10 us.
 - the loop is di-outer / q-inner and units are fused in QUARTET PAIRS:
   all tensor ops are [128, 1600] wide (half the op overhead), outputs
   pack per pair into one [128, 6400] tile DMA'd as 2x[64, .] pieces.
 - weights pack into ONE [128, 6144] tile ([w | w*a]), coefs into 4
   per-dtile [128, 1601] f32 tiles ([a^2-pattern | b^2-pattern | ab]).
Engine split: PE matmuls; ACT stages PSUM->SBUF and computes
tmp = ab*syn_odd_shifted (per-partition scale); gpsimd does the two
tensor_adds (1.7us/800col contiguous); DVE does 4x800-col scans + two
fp16 2x subs per pair. PSUM: one [128, 4096] tile = all 8 banks,
planes at 1024-col pitch per quartet (matmul outs may not cross banks).
"""

import os
from contextlib import ExitStack

import numpy as np

import concourse.bass as bass
import concourse.tile as tile
from concourse import bacc, mybir
from concourse import bass_utils

B, T, C, D = 512, 100, 700, 512
NCORES = 8
BC = B // NCORES  # 64 batches per core
NQ = 4
K = 50  # radix-2 pairs per sequence
UO = 16 * K  # cols per unit/quartet (800)
PW = 2 * UO  # pair width (1600)
OTW = 4 * PW  # packed out tile width per pair (6400)
KT = [(k * 128, min(128, C - k * 128)) for k in range(6)]
F32 = mybir.dt.float32
FP16 = mybir.dt.float16
MULT = mybir.AluOpType.mult
ADD = mybir.AluOpType.add
COPY = mybir.ActivationFunctionType.Copy

MODE = "radix2-wfold-v5"
LAST_RESULT = None
_cache = {}


def _build():
    key = ("nc",)
    if key in _cache:
        return _cache[key]
    nc = bacc.Bacc("TRN2", target_bir_lowering=False, debug=False)

    # x per ktile: [q0e|q0o|q1e|q1o|...] 800-col blocks
    x_d = nc.dram_tensor("x16", [6, 128, 2 * BC * K], FP16, kind="ExternalInput").ap()
    w_d = nc.dram_tensor("w16", [128, 12 * D], FP16, kind="ExternalInput").ap()
    cf_d = nc.dram_tensor("coefs", [4, 128, PW + 1], F32, kind="ExternalInput").ap()
    out_d = nc.dram_tensor("out", [D, 2 * OTW], FP16, kind="ExternalOutput").ap()

    QS = None  # round-robin DMA queue list, set inside context

    with tile.TileContext(nc) as tc:
        with ExitStack() as ctx:
            cpool = ctx.enter_context(tc.tile_pool(name="consts", bufs=1))
            warm_sb = cpool.tile([128, 512], FP16, name="warm", tag="warm")
            nc.gpsimd.memset(warm_sb[:], 0.0)

            QS = [nc.sync, nc.gpsimd, nc.scalar]
            qi = [0]

            def rr_dma(dst, src, rows=64, nq=3):
                # split into [rows, .] pieces, round-robin across queues
                p0 = 0
                P = dst.shape[0]
                while p0 < P:
                    p1 = min(p0 + rows, P)
                    QS[qi[0] % nq].dma_start(dst[p0:p1], src[p0:p1])
                    qi[0] += 1
                    p0 = p1

            # startup order: weights, then x, then coefs (first scan needs
            # coefs only after matmuls+staging of the first pair)
            w_all = cpool.tile([128, 12 * D], FP16, name="w_all", tag="w_all")
            rr_dma(w_all[:], w_d[:])
            x_ts = []
            for k in range(6):
                t_ = cpool.tile([128, 2 * BC * K], FP16, name=f"x{k}", tag=f"x{k}")
                rr_dma(t_[:], x_d[k])
                x_ts.append(t_)
            cf_t = []
            for di in range(4):
                t_ = cpool.tile([128, PW + 1], F32, name=f"cf{di}", tag=f"cf{di}")
                rr_dma(t_[:], cf_d[di])
                cf_t.append(t_)

            pp = ctx.enter_context(tc.tile_pool(name="pp", bufs=1, space="PSUM"))
            sp = ctx.enter_context(tc.tile_pool(name="sp", bufs=2))
            gp = ctx.enter_context(tc.tile_pool(name="gp", bufs=2))
            op = ctx.enter_context(tc.tile_pool(name="op", bufs=2))

            # PE warmup during the x DMA wait (p-state ramp)
            warm_ps = pp.tile([128, 4096], F32, tag="ps", name="warm_ps")
            for _ in range(10):
                nc.tensor.matmul(
                    warm_ps[:, 0:384], warm_sb[:, 0:128], warm_sb[:, 0:384],
                    start=True, stop=True,
                )

            state = {}

            def syn_part(di, qp, pidx):
                dsl = slice(di * 128, (di + 1) * 128)
                ps = pp.tile([128, 4096], F32, tag="ps", name=f"ps_{di}_{qp}")
                # plane regions: per quartet at 1024-col pitch
                for k, (r0_, rk) in enumerate(KT):
                    wo = 6 * D + k * D + di * 128  # w*a block
                    we = k * D + di * 128  # w block
                    for j, q in enumerate((2 * qp, 2 * qp + 1)):
                        xq = q * PW
                        for c0, c1 in ((0, 512), (512, UO)):
                            nc.tensor.matmul(
                                ps[:, j * 1024 + c0 : j * 1024 + c1],
                                w_all[:rk, wo : wo + 128],
                                x_ts[k][:rk, xq + UO + c0 : xq + UO + c1],
                                start=(k == 0), stop=(k == 5),
                            )
                    for j, q in enumerate((2 * qp, 2 * qp + 1)):
                        xq = q * PW
                        for c0, c1 in ((0, 512), (512, UO)):
                            nc.tensor.matmul(
                                ps[:, 2048 + j * 1024 + c0 : 2048 + j * 1024 + c1],
                                w_all[:rk, we : we + 128],
                                x_ts[k][:rk, xq + c0 : xq + c1],
                                start=(k == 0), stop=(k == 5),
                            )

                po3 = ps[:, 0:2048].rearrange("p (j c) -> p j c", c=1024)[:, :, 0:UO]
                pe3 = ps[:, 2048:4096].rearrange("p (j c) -> p j c", c=1024)[:, :, 0:UO]
                po16 = sp.tile([128, PW], FP16, tag="po16", name=f"po16_{pidx}")
                nc.scalar.activation(
                    po16[:].rearrange("p (j c) -> p j c", c=UO), po3, COPY
                )
                pe16 = sp.tile([128, PW], FP16, tag="pe16", name=f"pe16_{pidx}")
                nc.scalar.activation(
                    pe16[:].rearrange("p (j c) -> p j c", c=UO), pe3, COPY
                )

                ot = op.tile([128, OTW], FP16, tag="ot", name=f"ot_{pidx}")
                s_odd = ot[:, 0:PW]
                Se = ot[:, 2 * PW : 3 * PW]

                g = gp.tile([128, PW], FP16, tag="g", name=f"g_{pidx}")
                nc.gpsimd.tensor_add(g[:], po16[:], pe16[:])
                for h in range(2):
                    nc.vector.tensor_tensor_scan(
                        s_odd[:, h * UO : (h + 1) * UO],
                        cf_t[di][:, 0:UO],
                        g[:, h * UO : (h + 1) * UO],
                        0.0, MULT, ADD,
                    )
                nc.vector.tensor_sub(Se, s_odd, pe16[:])
                state[(di, qp)] = (ot, dsl)

            def mem_part(di, qp, pidx):
                ot, dsl = state.pop((di, qp))
                s_odd = ot[:, 0:PW]
                V = ot[:, PW : 2 * PW]
                Se = ot[:, 2 * PW : 3 * PW]
                Me = ot[:, 3 * PW : OTW]
                s3 = s_odd.rearrange("p (s c) -> p s c", c=K)

                tmp = sp.tile([128, PW], FP16, tag="tmp", name=f"tmp_{pidx}")
                t3 = tmp[:].rearrange("p (s c) -> p s c", c=K)
                if pidx < 2:
                    nc.gpsimd.memset(t3[:, :, 0:1], 0.0)
                nc.scalar.activation(
                    t3[:, :, 1:K], s3[:, :, 0 : K - 1], COPY,
                    scale=cf_t[di][:, PW : PW + 1],
                )

                gm = gp.tile([128, PW], FP16, tag="gm", name=f"gm_{pidx}")
                nc.gpsimd.tensor_add(gm[:], tmp[:], Se)
                for h in range(2):
                    nc.vector.tensor_tensor_scan(
                        V[:, h * UO : (h + 1) * UO],
                        cf_t[di][:, UO:PW],
                        gm[:, h * UO : (h + 1) * UO],
                        0.0, MULT, ADD,
                    )
                nc.vector.tensor_sub(Me, V, Se)

                # outputs avoid the vector queue (DVE engine time is precious)
                rr_dma(out_d[dsl, qp * OTW : (qp + 1) * OTW], ot[:], nq=3)

            pairs = [(di, qp) for di in range(4) for qp in range(2)]
            prev = None
            for pidx, (di, qp) in enumerate(pairs):
                syn_part(di, qp, pidx)
                if prev is not None:
                    mem_part(*prev, pidx - 1)
                prev = (di, qp)
            mem_part(*prev, len(pairs) - 1)

    nc.compile()
    _cache[key] = nc
    return nc


def kernel(inputs, w, alpha, beta):
    global LAST_RESULT
    inputs = np.asarray(inputs, dtype=np.float32)
    w = np.asarray(w, dtype=np.float32)
    alpha = np.asarray(alpha, dtype=np.float32).reshape(-1)
    beta = np.asarray(beta, dtype=np.float32).reshape(-1)

    nc = _build()

    def coef(sq):
        c = np.broadcast_to(sq.reshape(4, 128, 1), (4, 128, UO)).astype(np.float32).copy()
        c3 = c.reshape(4, 128, 16, K)
        c3[:, :, :, 0] = 0.0
        return c

    coefs = np.concatenate(
        [
            coef(alpha * alpha),
            coef(beta * beta),
            (alpha * beta).reshape(4, 128, 1).astype(np.float32),
        ],
        axis=2,
    )

    wpack = np.zeros((128, 12 * D), dtype=np.float16)
    wa = (w * alpha.reshape(1, D)).astype(np.float16)
    w16 = w.astype(np.float16)
    for k, (r0_, rk) in enumerate(KT):
        wpack[:rk, k * D : k * D + D] = w16[r0_ : r0_ + rk, :]
        wpack[:rk, 6 * D + k * D : 6 * D + k * D + D] = wa[r0_ : r0_ + rk, :]

    in_maps = []
    for c in range(NCORES):
        xc = inputs[c * BC : (c + 1) * BC]  # [64, 100, 700]
        xe = xc[:, 0::2, :]  # [64, 50, 700] (t = 0,2,..,98)
        xo = np.zeros((BC, K, C), dtype=np.float32)
        xo[:, 1:, :] = xc[:, 1:98:2, :]  # t = 1,3,..,97 -> k=1..49
        xe16 = xe.reshape(NQ, 16 * K, C).transpose(0, 2, 1)  # [q, 700, 800]
        xo16 = xo.reshape(NQ, 16 * K, C).transpose(0, 2, 1)
        xq = (
            np.concatenate([xe16, xo16], axis=2)  # [q, 700, 1600]
            .transpose(1, 0, 2)
            .reshape(C, 2 * BC * K)
            .astype(np.float16)
        )
        x16 = np.zeros((6, 128, 2 * BC * K), dtype=np.float16)
        for k, (r0_, rk) in enumerate(KT):
            x16[k, :rk, :] = xq[r0_ : r0_ + rk, :]
        in_maps.append({"x16": x16, "w16": wpack, "coefs": coefs})

    run_kwargs = {}
    if os.environ.get("MEMBRANE_TRACE_DIR"):
        run_kwargs["tmpdir"] = os.environ["MEMBRANE_TRACE_DIR"]
    res = bass_utils.run_bass_kernel_spmd(
        nc, in_maps, core_ids=list(range(NCORES)), **run_kwargs
    )
    LAST_RESULT = res

    inv_a = (1.0 / alpha).reshape(1, 1, D)
    mo_sc = ((1.0 - beta) / alpha).reshape(1, 1, D)
    me_sc = ((1.0 - beta) / (alpha * beta)).reshape(1, 1, D)

    syn_full = np.empty((B, T, D), dtype=np.float32)
    mem_full = np.empty((B, T, D), dtype=np.float32)
    for c in range(NCORES):
        r = res.results[c]["out"].astype(np.float32)  # [512, 2*OTW]
        for qp in range(2):
            blk = r[:, qp * OTW : (qp + 1) * OTW]
            so = blk[:, 0:PW].reshape(D, 2, 16, K)  # [d, j, s, k] t=2k+1
            V = blk[:, PW : 2 * PW].reshape(D, 2, 16, K)
            Sev = blk[:, 2 * PW : 3 * PW].reshape(D, 2, 16, K)
            Mev = blk[:, 3 * PW : OTW].reshape(D, 2, 16, K)
            for j in range(2):
                b0 = c * BC + (2 * qp + j) * 16
                syn_full[b0 : b0 + 16, 1::2, :] = so[:, j].transpose(1, 2, 0)
                syn_full[b0 : b0 + 16, 0::2, :] = Sev[:, j].transpose(1, 2, 0) * inv_a
                mem_full[b0 : b0 + 16, 1::2, :] = V[:, j].transpose(1, 2, 0) * mo_sc
                mem_full[b0 : b0 + 16, 0::2, :] = Mev[:, j].transpose(1, 2, 0) * me_sc
    return (syn_full, mem_full)


# revision 20
# speedup vs baseline: 1.1288x; 1.1288x over previous
"""Trainium2 Bass kernel for nn_MembraneLayer: h = x @ w followed by a
double first-order recurrence over time, producing (syn_rec, mem_rec).

Sharding: data-parallel over batch. 8 cores x 64 batches each.

Radix-2 "weight-folded" design (v6). DVE scans only ODD timesteps and
recovers evens algebraically:

  syn[2k+1] = a^2 syn[2k-1] + (a*h[2k-1] + h[2k])   <- scan over g
  a*syn[2k] = syn[2k+1] - h[2k]                     <- TT sub (fp16 2x)
  V[k] := a*v[2k+1] = b^2 V[k-1] + ab*syn[2k-1] + a*syn[2k]  <- scan g_m
  ab*v[2k]  = V[k] - a*syn[2k]                      <- TT sub
  (v = mem/(1-b); host applies the final per-channel scales)

Pair-combines fold into PE via host-premultiplied weights (w*a on odd
x cols -> Po plane with leading zero col, w on even -> Pe), keeping
matmul cost at baseline. gpsimd does the two tensor_adds (contiguous
2-D: 1.7us/800col measured), ACT stages PSUM->SBUF + per-partition
tmp = ab*syn_odd_shift, DVE does the 2 scans + 2 subs per unit.

DMA (measured): a queue sustains ~65 GB/s at 3200B rows and ~130 GB/s
at 6400B rows; a [128, W] DMA costs ~6.3us of queue time. So all
DRAM transfers use 6400B rows ([128, 3200-col fp16] or [128, 1601-col
f32]) and are round-robined over the 3 DGE queues (sync/gpsimd/
scalar): x as quartet-PAIR tiles (12 DMAs), outputs per unit (16),
weights packed in 2, coefs in 4. Emission is software-pipelined
(unit u's syn part, then unit u-1's mem part).
"""

import os
from contextlib import ExitStack

import numpy as np

import concourse.bass as bass
import concourse.tile as tile
from concourse import bacc, mybir
from concourse import bass_utils

B, T, C, D = 512, 100, 700, 512
NCORES = 8
BC = B // NCORES  # 64 batches per core
NQ = 4
K = 50  # radix-2 pairs per sequence
UO = 16 * K  # cols per unit/quartet (800)
OTW = 4 * UO  # packed out tile width (3200)
KT = [(k * 128, min(128, C - k * 128)) for k in range(6)]
F32 = mybir.dt.float32
FP16 = mybir.dt.float16
MULT = mybir.AluOpType.mult
ADD = mybir.AluOpType.add
COPY = mybir.ActivationFunctionType.Copy

MODE = "radix2-wfold-v6"
LAST_RESULT = None
_cache = {}


def _build():
    key = ("nc",)
    if key in _cache:
        return _cache[key]
    nc = bacc.Bacc("TRN2", target_bir_lowering=False, debug=False)

    # x per ktile: 4 quartet blocks of [xe 800 | xo 800]
    x_d = nc.dram_tensor("x16", [6, 128, NQ * 2 * UO], FP16, kind="ExternalInput").ap()
    w_d = nc.dram_tensor("w16", [128, 12 * D], FP16, kind="ExternalInput").ap()
    cf_d = nc.dram_tensor("coefs", [4, 128, 2 * UO + 1], F32, kind="ExternalInput").ap()
    out_d = nc.dram_tensor("out", [D, NQ * OTW], FP16, kind="ExternalOutput").ap()

    with tile.TileContext(nc) as tc:
        with ExitStack() as ctx:
            cpool = ctx.enter_context(tc.tile_pool(name="consts", bufs=1))
            warm_sb = cpool.tile([128, 512], FP16, name="warm", tag="warm")
            nc.gpsimd.memset(warm_sb[:], 0.0)

            QS = [nc.sync, nc.gpsimd, nc.scalar]
            qi = [0]

            def rr_dma(dst, src):
                QS[qi[0] % 3].dma_start(dst, src)
                qi[0] += 1

            # weights packed [w 6x512 | w*a 6x512], split as 2 DMAs
            w_all = cpool.tile([128, 12 * D], FP16, name="w_all", tag="w_all")
            rr_dma(w_all[:, 0 : 6 * D], w_d[:, 0 : 6 * D])
            rr_dma(w_all[:, 6 * D : 12 * D], w_d[:, 6 * D : 12 * D])
            cf_t = []
            for di in range(4):
                t_ = cpool.tile([128, 2 * UO + 1], F32, name=f"cf{di}", tag=f"cf{di}")
                rr_dma(t_[:], cf_d[di])
                cf_t.append(t_)

            xp = ctx.enter_context(tc.tile_pool(name="xp", bufs=2))
            pp = ctx.enter_context(tc.tile_pool(name="pp", bufs=2, space="PSUM"))
            sp = ctx.enter_context(tc.tile_pool(name="sp", bufs=3))
            gp = ctx.enter_context(tc.tile_pool(name="gp", bufs=3))
            op = ctx.enter_context(tc.tile_pool(name="op", bufs=3))

            warm_ps = pp.tile([128, 2048], F32, tag="ps", name="warm_ps")
            for _ in range(10):
                nc.tensor.matmul(
                    warm_ps[:, 0:384], warm_sb[:, 0:128], warm_sb[:, 0:384],
                    start=True, stop=True,
                )

            state = {}

            def syn_part(q, di):
                dsl = slice(di * 128, (di + 1) * 128)
                x_ts = state["x"]
                xoff = (q % 2) * 2 * UO  # quartet offset within the pair tile

                ps = pp.tile([128, 2048], F32, tag="ps", name=f"ps_{q}_{di}")
                po = ps[:, 0:UO]
                pe = ps[:, 1024 : 1024 + UO]
                for k, (r0_, rk) in enumerate(KT):
                    wo = 6 * D + k * D + di * 128
                    for c0, c1 in ((0, 512), (512, UO)):
                        nc.tensor.matmul(
                            po[:, c0:c1], w_all[:rk, wo : wo + 128],
                            x_ts[k][:rk, xoff + UO + c0 : xoff + UO + c1],
                            start=(k == 0), stop=(k == 5),
                        )
                for k, (r0_, rk) in enumerate(KT):
                    we = k * D + di * 128
                    for c0, c1 in ((0, 512), (512, UO)):
                        nc.tensor.matmul(
                            pe[:, c0:c1], w_all[:rk, we : we + 128],
                            x_ts[k][:rk, xoff + c0 : xoff + c1],
                            start=(k == 0), stop=(k == 5),
                        )

                po16 = sp.tile([128, UO], FP16, tag="po16", name=f"po16_{q}_{di}")
                nc.scalar.activation(po16[:], po, COPY)
                pe16 = sp.tile([128, UO], FP16, tag="pe16", name=f"pe16_{q}_{di}")
                nc.scalar.activation(pe16[:], pe, COPY)

                ot = op.tile([128, OTW], FP16, tag="ot", name=f"ot_{q}_{di}")
                s_odd = ot[:, 0:UO]
                Se = ot[:, 2 * UO : 3 * UO]

                g = gp.tile([128, UO], FP16, tag="g", name=f"g_{q}_{di}")
                nc.gpsimd.tensor_add(g[:], po16[:], pe16[:])
                nc.vector.tensor_tensor_scan(
                    s_odd, cf_t[di][:, 0:UO], g[:], 0.0, MULT, ADD
                )
                nc.vector.tensor_sub(Se, s_odd, pe16[:])
                state[(q, di)] = (ot, dsl)

            def mem_part(q, di, mcall):
                ot, dsl = state.pop((q, di))
                s_odd = ot[:, 0:UO]
                V = ot[:, UO : 2 * UO]
                Se = ot[:, 2 * UO : 3 * UO]
                Me = ot[:, 3 * UO : OTW]
                s3 = s_odd.rearrange("p (s c) -> p s c", c=K)

                tmp = sp.tile([128, UO], FP16, tag="tmp", name=f"tmp_{q}_{di}")
                t3 = tmp[:].rearrange("p (s c) -> p s c", c=K)
                if mcall < 3:
                    nc.gpsimd.memset(t3[:, :, 0:1], 0.0)
                nc.scalar.activation(
                    t3[:, :, 1:K], s3[:, :, 0 : K - 1], COPY,
                    scale=cf_t[di][:, 2 * UO : 2 * UO + 1],
                )

                gm = gp.tile([128, UO], FP16, tag="gm", name=f"gm_{q}_{di}")
                nc.gpsimd.tensor_add(gm[:], tmp[:], Se)
                nc.vector.tensor_tensor_scan(
                    V, cf_t[di][:, UO : 2 * UO], gm[:], 0.0, MULT, ADD
                )
                nc.vector.tensor_sub(Me, V, Se)

                rr_dma(out_d[dsl, q * OTW : (q + 1) * OTW], ot[:])

            prev = None
            mcall = 0
            for q in range(NQ):
                if q % 2 == 0:
                    x_ts = []
                    pc0 = q * 2 * UO  # pair covers quartets q, q+1
                    for k in range(6):
                        t_ = xp.tile(
                            [128, 4 * UO], FP16, tag=f"x{k}", name=f"x{k}_{q}"
                        )
                        rr_dma(t_[:], x_d[k][:, pc0 : pc0 + 4 * UO])
                        x_ts.append(t_)
                    state["x"] = x_ts
                for di in range(4):
                    syn_part(q, di)
                    if prev is not None:
                        mem_part(*prev, mcall)
                        mcall += 1
                    prev = (q, di)
            mem_part(*prev, mcall)

    nc.compile()
    _cache[key] = nc
    return nc


def kernel(inputs, w, alpha, beta):
    global LAST_RESULT
    inputs = np.asarray(inputs, dtype=np.float32)
    w = np.asarray(w, dtype=np.float32)
    alpha = np.asarray(alpha, dtype=np.float32).reshape(-1)
    beta = np.asarray(beta, dtype=np.float32).reshape(-1)

    nc = _build()

    def coef(sq):
        c = np.broadcast_to(sq.reshape(4, 128, 1), (4, 128, UO)).astype(np.float32).copy()
        c3 = c.reshape(4, 128, 16, K)
        c3[:, :, :, 0] = 0.0
        return c

    coefs = np.concatenate(
        [
            coef(alpha * alpha),
            coef(beta * beta),
            (alpha * beta).reshape(4, 128, 1).astype(np.float32),
        ],
        axis=2,
    )

    wpack = np.zeros((128, 12 * D), dtype=np.float16)
    wa = (w * alpha.reshape(1, D)).astype(np.float16)
    w16 = w.astype(np.float16)
    for k, (r0_, rk) in enumerate(KT):
        wpack[:rk, k * D : k * D + D] = w16[r0_ : r0_ + rk, :]
        wpack[:rk, 6 * D + k * D : 6 * D + k * D + D] = wa[r0_ : r0_ + rk, :]

    in_maps = []
    for c in range(NCORES):
        xc = inputs[c * BC : (c + 1) * BC]  # [64, 100, 700]
        xe = xc[:, 0::2, :]  # [64, 50, 700] (t = 0,2,..,98)
        xo = np.zeros((BC, K, C), dtype=np.float32)
        xo[:, 1:, :] = xc[:, 1:98:2, :]  # t = 1,3,..,97 -> k=1..49
        xe16 = xe.reshape(NQ, 16 * K, C).transpose(0, 2, 1)  # [q, 700, 800]
        xo16 = xo.reshape(NQ, 16 * K, C).transpose(0, 2, 1)
        xq = (
            np.concatenate([xe16, xo16], axis=2)  # [q, 700, 1600]
            .transpose(1, 0, 2)
            .reshape(C, NQ * 2 * UO)
            .astype(np.float16)
        )
        x16 = np.zeros((6, 128, NQ * 2 * UO), dtype=np.float16)
        for k, (r0_, rk) in enumerate(KT):
            x16[k, :rk, :] = xq[r0_ : r0_ + rk, :]
        in_maps.append({"x16": x16, "w16": wpack, "coefs": coefs})

    run_kwargs = {}
    if os.environ.get("MEMBRANE_TRACE_DIR"):
        run_kwargs["tmpdir"] = os.environ["MEMBRANE_TRACE_DIR"]
    res = bass_utils.run_bass_kernel_spmd(
        nc, in_maps, core_ids=list(range(NCORES)), **run_kwargs
    )
    LAST_RESULT = res

    inv_a = (1.0 / alpha).reshape(1, 1, D)
    mo_sc = ((1.0 - beta) / alpha).reshape(1, 1, D)
    me_sc = ((1.0 - beta) / (alpha * beta)).reshape(1, 1, D)

    syn_full = np.empty((B, T, D), dtype=np.float32)
    mem_full = np.empty((B, T, D), dtype=np.float32)
    for c in range(NCORES):
        r = res.results[c]["out"].astype(np.float32)  # [512, NQ*OTW]
        for q in range(NQ):
            blk = r[:, q * OTW : (q + 1) * OTW]
            so = blk[:, 0:UO].reshape(D, 16, K)  # t=2k+1
            V = blk[:, UO : 2 * UO].reshape(D, 16, K)
            Sev = blk[:, 2 * UO : 3 * UO].reshape(D, 16, K)
            Mev = blk[:, 3 * UO : OTW].reshape(D, 16, K)
            b0 = c * BC + q * 16
            syn_full[b0 : b0 + 16, 1::2, :] = so.transpose(1, 2, 0)
            syn_full[b0 : b0 + 16, 0::2, :] = Sev.transpose(1, 2, 0) * inv_a
            mem_full[b0 : b0 + 16, 1::2, :] = V.transpose(1, 2, 0) * mo_sc
            mem_full[b0 : b0 + 16, 0::2, :] = Mev.transpose(1, 2, 0) * me_sc
    return (syn_full, mem_full)


# revision 23
# speedup vs baseline: 1.2062x; 1.0686x over previous
"""Trainium2 Bass kernel for nn_MembraneLayer: h = x @ w followed by a
double first-order recurrence over time, producing (syn_rec, mem_rec).

Sharding: data-parallel over batch. 8 cores x 64 batches each.

Radix-2 "weight-folded" design (v6). DVE scans only ODD timesteps and
recovers evens algebraically:

  syn[2k+1] = a^2 syn[2k-1] + (a*h[2k-1] + h[2k])   <- scan over g
  a*syn[2k] = syn[2k+1] - h[2k]                     <- TT sub (fp16 2x)
  V[k] := a*v[2k+1] = b^2 V[k-1] + ab*syn[2k-1] + a*syn[2k]  <- scan g_m
  ab*v[2k]  = V[k] - a*syn[2k]                      <- TT sub
  (v = mem/(1-b); host applies the final per-channel scales)

Pair-combines fold into PE via host-premultiplied weights (w*a on odd
x cols -> Po plane with leading zero col, w on even -> Pe), keeping
matmul cost at baseline. gpsimd does the two tensor_adds (contiguous
2-D: 1.7us/800col measured), ACT stages PSUM->SBUF + per-partition
tmp = ab*syn_odd_shift, DVE does the 2 scans + 2 subs per unit.

DMA (measured): a queue sustains ~65 GB/s at 3200B rows and ~130 GB/s
at 6400B rows; a [128, W] DMA costs ~6.3us of queue time. So all
DRAM transfers use 6400B rows ([128, 3200-col fp16] or [128, 1601-col
f32]) and are round-robined over the 3 DGE queues (sync/gpsimd/
scalar): x as quartet-PAIR tiles (12 DMAs), outputs per unit (16),
weights packed in 2, coefs in 4. Emission is software-pipelined
(unit u's syn part, then unit u-1's mem part).
"""

import os
from contextlib import ExitStack

import numpy as np

import concourse.bass as bass
import concourse.tile as tile
from concourse import bacc, mybir
from concourse import bass_utils

B, T, C, D = 512, 100, 700, 512
NCORES = 8
BC = B // NCORES  # 64 batches per core
NQ = 4
K = 50  # radix-2 pairs per sequence
UO = 16 * K  # cols per unit/quartet (800)
OTW = 4 * UO  # packed out tile width (3200)
KT = [(k * 128, min(128, C - k * 128)) for k in range(6)]
F32 = mybir.dt.float32
FP16 = mybir.dt.float16
MULT = mybir.AluOpType.mult
ADD = mybir.AluOpType.add
COPY = mybir.ActivationFunctionType.Copy

MODE = "radix2-wfold-v6"
LAST_RESULT = None
_cache = {}


def _build():
    key = ("nc",)
    if key in _cache:
        return _cache[key]
    nc = bacc.Bacc("TRN2", target_bir_lowering=False, debug=False)

    # x per ktile: 4 quartet blocks of [xe 800 | xo 800]
    x_d = nc.dram_tensor("x16", [6, 128, NQ * 2 * UO], FP16, kind="ExternalInput").ap()
    w_d = nc.dram_tensor("w16", [128, 12 * D], FP16, kind="ExternalInput").ap()
    cf_d = nc.dram_tensor("coefs", [4, 128, 2 * UO + 1], F32, kind="ExternalInput").ap()
    out_d = nc.dram_tensor("out", [D, NQ * OTW], FP16, kind="ExternalOutput").ap()

    with tile.TileContext(nc) as tc:
        with ExitStack() as ctx:
            cpool = ctx.enter_context(tc.tile_pool(name="consts", bufs=1))
            warm_sb = cpool.tile([128, 512], FP16, name="warm", tag="warm")
            nc.gpsimd.memset(warm_sb[:], 0.0)

            QS = [nc.sync, nc.gpsimd, nc.scalar]
            qi = [0]

            def rr_dma(dst, src, qs=None):
                qs = QS if qs is None else qs
                qs[qi[0] % len(qs)].dma_start(dst, src)
                qi[0] += 1

            # weights packed [w 6x512 | w*a 6x512]: 4 x [64-row] pieces on
            # the two HWDGE queues so they land before the first x tiles
            w_all = cpool.tile([128, 12 * D], FP16, name="w_all", tag="w_all")
            for p0 in (0, 64):
                for h0 in (0, 6 * D):
                    rr_dma(
                        w_all[p0 : p0 + 64, h0 : h0 + 6 * D],
                        w_d[p0 : p0 + 64, h0 : h0 + 6 * D],
                        qs=[nc.sync, nc.scalar],
                    )

            xp = ctx.enter_context(tc.tile_pool(name="xp", bufs=2))
            pp = ctx.enter_context(tc.tile_pool(name="pp", bufs=2, space="PSUM"))
            sp = ctx.enter_context(tc.tile_pool(name="sp", bufs=3))
            gp = ctx.enter_context(tc.tile_pool(name="gp", bufs=3))
            op = ctx.enter_context(tc.tile_pool(name="op", bufs=3))

            warm_ps = pp.tile([128, 2048], F32, tag="ps", name="warm_ps")
            for _ in range(10):
                nc.tensor.matmul(
                    warm_ps[:, 0:384], warm_sb[:, 0:128], warm_sb[:, 0:384],
                    start=True, stop=True,
                )

            state = {}

            def syn_part(q, di):
                dsl = slice(di * 128, (di + 1) * 128)
                x_ts = state["x"]
                xoff = (q % 2) * 2 * UO  # quartet offset within the pair tile

                ps = pp.tile([128, 2048], F32, tag="ps", name=f"ps_{q}_{di}")
                po = ps[:, 0:UO]
                pe = ps[:, 1024 : 1024 + UO]
                for k, (r0_, rk) in enumerate(KT):
                    wo = 6 * D + k * D + di * 128
                    for c0, c1 in ((0, 512), (512, UO)):
                        nc.tensor.matmul(
                            po[:, c0:c1], w_all[:rk, wo : wo + 128],
                            x_ts[k][:rk, xoff + UO + c0 : xoff + UO + c1],
                            start=(k == 0), stop=(k == 5),
                        )
                for k, (r0_, rk) in enumerate(KT):
                    we = k * D + di * 128
                    for c0, c1 in ((0, 512), (512, UO)):
                        nc.tensor.matmul(
                            pe[:, c0:c1], w_all[:rk, we : we + 128],
                            x_ts[k][:rk, xoff + c0 : xoff + c1],
                            start=(k == 0), stop=(k == 5),
                        )

                po16 = sp.tile([128, UO], FP16, tag="po16", name=f"po16_{q}_{di}")
                nc.scalar.activation(po16[:], po, COPY)
                pe16 = sp.tile([128, UO], FP16, tag="pe16", name=f"pe16_{q}_{di}")
                nc.scalar.activation(pe16[:], pe, COPY)

                ot = op.tile([128, OTW], FP16, tag="ot", name=f"ot_{q}_{di}")
                s_odd = ot[:, 0:UO]
                Se = ot[:, 2 * UO : 3 * UO]

                g = gp.tile([128, UO], FP16, tag="g", name=f"g_{q}_{di}")
                nc.gpsimd.tensor_add(g[:], po16[:], pe16[:])
                nc.vector.tensor_tensor_scan(
                    s_odd, cf_t[di][:, 0:UO], g[:], 0.0, MULT, ADD
                )
                nc.vector.tensor_sub(Se, s_odd, pe16[:])
                state[(q, di)] = (ot, dsl)

            def mem_part(q, di, mcall):
                ot, dsl = state.pop((q, di))
                s_odd = ot[:, 0:UO]
                V = ot[:, UO : 2 * UO]
                Se = ot[:, 2 * UO : 3 * UO]
                Me = ot[:, 3 * UO : OTW]
                s3 = s_odd.rearrange("p (s c) -> p s c", c=K)

                tmp = sp.tile([128, UO], FP16, tag="tmp", name=f"tmp_{q}_{di}")
                t3 = tmp[:].rearrange("p (s c) -> p s c", c=K)
                if mcall < 3:
                    nc.gpsimd.memset(t3[:, :, 0:1], 0.0)
                nc.scalar.activation(
                    t3[:, :, 1:K], s3[:, :, 0 : K - 1], COPY,
                    scale=cf_t[di][:, 2 * UO : 2 * UO + 1],
                )

                gm = gp.tile([128, UO], FP16, tag="gm", name=f"gm_{q}_{di}")
                nc.gpsimd.tensor_add(gm[:], tmp[:], Se)
                nc.vector.tensor_tensor_scan(
                    V, cf_t[di][:, UO : 2 * UO], gm[:], 0.0, MULT, ADD
                )
                nc.vector.tensor_sub(Me, V, Se)

                # outputs on the two HWDGE queues (keep SWDGE issuance and
                # the gpsimd engine clear for the adds)
                rr_dma(out_d[dsl, q * OTW : (q + 1) * OTW], ot[:],
                       qs=[nc.sync, nc.scalar])

            cf_t = [
                cpool.tile([128, 2 * UO + 1], F32, name=f"cf{di}", tag=f"cf{di}")
                for di in range(4)
            ]
            prev = None
            mcall = 0
            for q in range(NQ):
                if q % 2 == 0:
                    x_ts = []
                    pc0 = q * 2 * UO  # pair covers quartets q, q+1
                    for k in range(6):
                        t_ = xp.tile(
                            [128, 4 * UO], FP16, tag=f"x{k}", name=f"x{k}_{q}"
                        )
                        rr_dma(t_[:], x_d[k][:, pc0 : pc0 + 4 * UO])
                        x_ts.append(t_)
                    state["x"] = x_ts
                    if q == 0:
                        # coefs after the first x pair: needed only once the
                        # first pair's matmuls+stagings+add are done
                        for di in range(4):
                            rr_dma(cf_t[di][:], cf_d[di])
                for di in range(4):
                    syn_part(q, di)
                    if prev is not None:
                        mem_part(*prev, mcall)
                        mcall += 1
                    prev = (q, di)
            mem_part(*prev, mcall)

    nc.compile()
    _cache[key] = nc
    return nc


def kernel(inputs, w, alpha, beta):
    global LAST_RESULT
    inputs = np.asarray(inputs, dtype=np.float32)
    w = np.asarray(w, dtype=np.float32)
    alpha = np.asarray(alpha, dtype=np.float32).reshape(-1)
    beta = np.asarray(beta, dtype=np.float32).reshape(-1)

    nc = _build()

    def coef(sq):
        c = np.broadcast_to(sq.reshape(4, 128, 1), (4, 128, UO)).astype(np.float32).copy()
        c3 = c.reshape(4, 128, 16, K)
        c3[:, :, :, 0] = 0.0
        return c

    coefs = np.concatenate(
        [
            coef(alpha * alpha),
            coef(beta * beta),
            (alpha * beta).reshape(4, 128, 1).astype(np.float32),
        ],
        axis=2,
    )

    wpack = np.zeros((128, 12 * D), dtype=np.float16)
    wa = (w * alpha.reshape(1, D)).astype(np.float16)
    w16 = w.astype(np.float16)
    for k, (r0_, rk) in enumerate(KT):
        wpack[:rk, k * D : k * D + D] = w16[r0_ : r0_ + rk, :]
        wpack[:rk, 6 * D + k * D : 6 * D + k * D + D] = wa[r0_ : r0_ + rk, :]

    in_maps = []
    for c in range(NCORES):
        xc = inputs[c * BC : (c + 1) * BC]  # [64, 100, 700]
        xe = xc[:, 0::2, :]  # [64, 50, 700] (t = 0,2,..,98)
        xo = np.zeros((BC, K, C), dtype=np.float32)
        xo[:, 1:, :] = xc[:, 1:98:2, :]  # t = 1,3,..,97 -> k=1..49
        xe16 = xe.reshape(NQ, 16 * K, C).transpose(0, 2, 1)  # [q, 700, 800]
        xo16 = xo.reshape(NQ, 16 * K, C).transpose(0, 2, 1)
        xq = (
            np.concatenate([xe16, xo16], axis=2)  # [q, 700, 1600]
            .transpose(1, 0, 2)
            .reshape(C, NQ * 2 * UO)
            .astype(np.float16)
        )
        x16 = np.zeros((6, 128, NQ * 2 * UO), dtype=np.float16)
        for k, (r0_, rk) in enumerate(KT):
            x16[k, :rk, :] = xq[r0_ : r0_ + rk, :]
        in_maps.append({"x16": x16, "w16": wpack, "coefs": coefs})

    run_kwargs = {}
    if os.environ.get("MEMBRANE_TRACE_DIR"):
        run_kwargs["tmpdir"] = os.environ["MEMBRANE_TRACE_DIR"]
    res = bass_utils.run_bass_kernel_spmd(
        nc, in_maps, core_ids=list(range(NCORES)), **run_kwargs
    )
    LAST_RESULT = res

    inv_a = (1.0 / alpha).reshape(1, 1, D)
    mo_sc = ((1.0 - beta) / alpha).reshape(1, 1, D)
    me_sc = ((1.0 - beta) / (alpha * beta)).reshape(1, 1, D)

    syn_full = np.empty((B, T, D), dtype=np.float32)
    mem_full = np.empty((B, T, D), dtype=np.float32)
    for c in range(NCORES):
        r = res.results[c]["out"].astype(np.float32)  # [512, NQ*OTW]
        for q in range(NQ):
            blk = r[:, q * OTW : (q + 1) * OTW]
            so = blk[:, 0:UO].reshape(D, 16, K)  # t=2k+1
            V = blk[:, UO : 2 * UO].reshape(D, 16, K)
            Sev = blk[:, 2 * UO : 3 * UO].reshape(D, 16, K)
            Mev = blk[:, 3 * UO : OTW].reshape(D, 16, K)
            b0 = c * BC + q * 16
            syn_full[b0 : b0 + 16, 1::2, :] = so.transpose(1, 2, 0)
            syn_full[b0 : b0 + 16, 0::2, :] = Sev.transpose(1, 2, 0) * inv_a
            mem_full[b0 : b0 + 16, 1::2, :] = V.transpose(1, 2, 0) * mo_sc
            mem_full[b0 : b0 + 16, 0::2, :] = Mev.transpose(1, 2, 0) * me_sc
    return (syn_full, mem_full)


# revision 24
# speedup vs baseline: 1.5289x; 1.2675x over previous
"""Trainium2 Bass kernel for nn_MembraneLayer: h = x @ w followed by a
double first-order recurrence over time, producing (syn_rec, mem_rec).

Sharding: data-parallel over batch. 8 cores x 64 batches each.

Radix-2 "weight-folded" design (v3). The DVE serial scan (~2.2 ns/col,
no perf modes) was the baseline bottleneck at 2x1600 scan cols per
(quartet, d_tile) unit. This version halves the scanned columns by
scanning only ODD timesteps and recovering the evens algebraically:

  syn[2k+1] = a^2 syn[2k-1] + (a*h[2k-1] + h[2k])   <- scan over g
  a*syn[2k] = syn[2k+1] - h[2k]                     <- TT sub (fp16 2x)
  V[k] := a*v[2k+1] = b^2 V[k-1] + ab*syn[2k-1] + a*syn[2k]  <- scan g_m
  ab*v[2k]  = V[k] - a*syn[2k]                      <- TT sub
  (v = mem/(1-b); host applies the final per-channel scales)

The pair-combines fold into the PE: host pre-multiplies weight copies
(w*a for odd-t columns, w for even-t), so PE emits the planes
  Po = a*h_odd (x odd cols, leading zero col for t=-1)
  Pe = h_even
at the SAME total matmul column count as the baseline. ACT stages
PSUM->SBUF and computes tmp = (ab)*syn_odd_shifted (per-partition
scale); the g/g_m adds and the two subs are fp16 2x-mode DVE ops
(gpsimd's Q7 software adds measured 3.5-5us/800col strided).

All tiles contiguous [128, 800]; sequence resets via coef=0 at k=0
cols; the shifted read's "syn[-1]=0" comes from a once-per-buffer
zeroed tmp column. Emission is software-pipelined: unit u's syn part
is followed by unit u-1's mem part so ACT's tmp has a full half-unit
to land before the DVE needs g_m.

Outputs per unit are packed in ONE [128, 3200] tile (s_odd | V | Se |
Me) -> single DMA. Host interleaves parities and applies per-channel
scales 1/a, (1-b)/a, (1-b)/(ab).

Measured: 137.5 us (baseline plain-scan kernel: 143.3 us on the same
harness). Restructured v4-v7 variants (fat-row DMAs, 3-queue round-
robin, gpsimd adds) all regressed to 148-210 us: per-queue DMA
dispatch (~25-50 ns/partition-row packet) makes input feed granularity
vs startup latency a hard tradeoff, and gpsimd/ACT offloads stretched
the critical chain. Keeping the proven config.
"""

import os
from contextlib import ExitStack

import numpy as np

import concourse.bass as bass
import concourse.tile as tile
from concourse import bacc, mybir
from concourse import bass_utils

B, T, C, D = 512, 100, 700, 512
NCORES = 8
BC = B // NCORES  # 64 batches per core
NQ = 4  # quartets: 16 batches each
K = 50  # radix-2 pairs per sequence
UO = 16 * K  # cols per unit (800)
OTW = 4 * UO  # packed out tile width (3200)
KT = [(k * 128, min(128, C - k * 128)) for k in range(6)]
F32 = mybir.dt.float32
FP16 = mybir.dt.float16
MULT = mybir.AluOpType.mult
ADD = mybir.AluOpType.add
COPY = mybir.ActivationFunctionType.Copy

MODE = "radix2-wfold-v3"
LAST_RESULT = None
_cache = {}


def _build():
    key = ("nc",)
    if key in _cache:
        return _cache[key]
    nc = bacc.Bacc("TRN2", target_bir_lowering=False, debug=False)

    xe_d = nc.dram_tensor("xe16", [C, BC * K], FP16, kind="ExternalInput").ap()
    xo_d = nc.dram_tensor("xo16", [C, BC * K], FP16, kind="ExternalInput").ap()
    w_d = nc.dram_tensor("w16", [C, D], FP16, kind="ExternalInput").ap()
    wa_d = nc.dram_tensor("wa16", [C, D], FP16, kind="ExternalInput").ap()
    # combined per-dtile coefs: [acoef 800 | bcoef 800 | abscale 1]
    cf_d = nc.dram_tensor("coefs", [4, 128, 2 * UO + 1], F32, kind="ExternalInput").ap()
    out_d = nc.dram_tensor("out", [D, NQ * OTW], FP16, kind="ExternalOutput").ap()

    with tile.TileContext(nc) as tc:
        with ExitStack() as ctx:
            cpool = ctx.enter_context(tc.tile_pool(name="consts", bufs=1))
            warm_sb = cpool.tile([128, 512], FP16, name="warm", tag="warm")
            nc.gpsimd.memset(warm_sb[:], 0.0)
            # weights on the gpsimd (SWDGE) queue; coefs on the scalar queue
            # (ACT is idle until the first staging) so the Sync queue leads
            # with the first x tiles and gpsimd's queue stays shallow
            w_tiles, wa_tiles = [], []
            for k, (r0_, rk) in enumerate(KT):
                wt = cpool.tile([128, D], FP16, name=f"w{k}", tag=f"w{k}")
                nc.gpsimd.dma_start(wt[:rk, :], w_d[r0_ : r0_ + rk, :])
                w_tiles.append(wt)
                wat = cpool.tile([128, D], FP16, name=f"wa{k}", tag=f"wa{k}")
                nc.gpsimd.dma_start(wat[:rk, :], wa_d[r0_ : r0_ + rk, :])
                wa_tiles.append(wat)
            cf_t = []
            for di in range(4):
                t_ = cpool.tile([128, 2 * UO + 1], F32, name=f"cf{di}", tag=f"cf{di}")
                nc.scalar.dma_start(t_[:], cf_d[di])
                cf_t.append(t_)

            xp = ctx.enter_context(tc.tile_pool(name="xp", bufs=2))
            pp = ctx.enter_context(tc.tile_pool(name="pp", bufs=2, space="PSUM"))
            sp = ctx.enter_context(tc.tile_pool(name="sp", bufs=3))
            gp = ctx.enter_context(tc.tile_pool(name="gp", bufs=3))
            op = ctx.enter_context(tc.tile_pool(name="op", bufs=3))

            # PE warmup: enough to trigger the p-state ramp without
            # overshooting the initial DMA wait
            warm_ps = pp.tile([128, 2048], F32, tag="ps", name="warm_ps")
            for _ in range(10):
                nc.tensor.matmul(
                    warm_ps[:, 0:384], warm_sb[:, 0:128], warm_sb[:, 0:384],
                    start=True, stop=True,
                )

            state = {}

            def syn_part(u):
                q, di = u
                dsl = slice(di * 128, (di + 1) * 128)
                xe_ts, xo_ts = state["x"]

                ps = pp.tile([128, 2048], F32, tag="ps", name=f"ps_{q}_{di}")
                po = ps[:, 0:UO]
                pe = ps[:, 1024 : 1024 + UO]
                # matmul outputs may not cross PSUM bank boundaries
                for k, (r0_, rk) in enumerate(KT):
                    for c0, c1 in ((0, 512), (512, UO)):
                        nc.tensor.matmul(
                            po[:, c0:c1], wa_tiles[k][:rk, dsl],
                            xo_ts[k][:rk, c0:c1],
                            start=(k == 0), stop=(k == 5),
                        )
                for k, (r0_, rk) in enumerate(KT):
                    for c0, c1 in ((0, 512), (512, UO)):
                        nc.tensor.matmul(
                            pe[:, c0:c1], w_tiles[k][:rk, dsl],
                            xe_ts[k][:rk, c0:c1],
                            start=(k == 0), stop=(k == 5),
                        )

                po16 = sp.tile([128, UO], FP16, tag="po16", name=f"po16_{q}_{di}")
                nc.scalar.activation(po16[:], po, COPY)
                pe16 = sp.tile([128, UO], FP16, tag="pe16", name=f"pe16_{q}_{di}")
                nc.scalar.activation(pe16[:], pe, COPY)

                ot = op.tile([128, OTW], FP16, tag="ot", name=f"ot_{q}_{di}")
                s_odd = ot[:, 0:UO]
                Se = ot[:, 2 * UO : 3 * UO]

                g = gp.tile([128, UO], FP16, tag="g", name=f"g_{q}_{di}")
                nc.vector.tensor_add(g[:], po16[:], pe16[:])
                nc.vector.tensor_tensor_scan(
                    s_odd, cf_t[di][:, 0:UO], g[:], 0.0, MULT, ADD
                )
                nc.vector.tensor_sub(Se, s_odd, pe16[:])
                state[u] = (ot, dsl)

            def mem_part(u, mcall):
                q, di = u
                ot, dsl = state.pop(u)
                s_odd = ot[:, 0:UO]
                V = ot[:, UO : 2 * UO]
                Se = ot[:, 2 * UO : 3 * UO]
                Me = ot[:, 3 * UO : OTW]
                s3 = s_odd.rearrange("p (s c) -> p s c", c=K)

                # tmp = (a*b) * syn[2k-1]; col k=0 per seq must be 0 -
                # zeroed once per pool buffer (ACT never writes col 0)
                tmp = sp.tile([128, UO], FP16, tag="tmp", name=f"tmp_{q}_{di}")
                t3 = tmp[:].rearrange("p (s c) -> p s c", c=K)
                if mcall < 3:
                    nc.gpsimd.memset(t3[:, :, 0:1], 0.0)
                nc.scalar.activation(
                    t3[:, :, 1:K], s3[:, :, 0 : K - 1], COPY,
                    scale=cf_t[di][:, 2 * UO : 2 * UO + 1],
                )

                gm = gp.tile([128, UO], FP16, tag="gm", name=f"gm_{q}_{di}")
                nc.vector.tensor_add(gm[:], tmp[:], Se)
                nc.vector.tensor_tensor_scan(
                    V, cf_t[di][:, UO : 2 * UO], gm[:], 0.0, MULT, ADD
                )
                nc.vector.tensor_sub(Me, V, Se)

                nc.scalar.dma_start(out_d[dsl, q * OTW : (q + 1) * OTW], ot[:])

            units = [(q, di) for q in range(NQ) for di in range(4)]
            prev = None
            mcall = 0
            for u in units:
                q, di = u
                if di == 0:
                    xe_ts, xo_ts = [], []
                    qc0 = q * UO
                    for k, (r0_, rk) in enumerate(KT):
                        te = xp.tile([128, UO], FP16, tag=f"xe{k}", name=f"xe{k}_{q}")
                        nc.sync.dma_start(
                            te[:rk, :], xe_d[r0_ : r0_ + rk, qc0 : qc0 + UO]
                        )
                        xe_ts.append(te)
                        to = xp.tile([128, UO], FP16, tag=f"xo{k}", name=f"xo{k}_{q}")
                        nc.sync.dma_start(
                            to[:rk, :], xo_d[r0_ : r0_ + rk, qc0 : qc0 + UO]
                        )
                        xo_ts.append(to)
                    state["x"] = (xe_ts, xo_ts)
                syn_part(u)
                if prev is not None:
                    mem_part(prev, mcall)
                    mcall += 1
                prev = u
            mem_part(prev, mcall)

    nc.compile()
    _cache[key] = nc
    return nc


def kernel(inputs, w, alpha, beta):
    global LAST_RESULT
    inputs = np.asarray(inputs, dtype=np.float32)
    w = np.asarray(w, dtype=np.float32)
    alpha = np.asarray(alpha, dtype=np.float32).reshape(-1)
    beta = np.asarray(beta, dtype=np.float32).reshape(-1)

    nc = _build()

    def coef(sq):
        c = np.broadcast_to(sq.reshape(4, 128, 1), (4, 128, UO)).astype(np.float32).copy()
        c3 = c.reshape(4, 128, 16, K)
        c3[:, :, :, 0] = 0.0
        return c

    coefs = np.concatenate(
        [
            coef(alpha * alpha),
            coef(beta * beta),
            (alpha * beta).reshape(4, 128, 1).astype(np.float32),
        ],
        axis=2,
    )
    w16 = w.astype(np.float16)
    wa16 = (w * alpha.reshape(1, D)).astype(np.float16)

    in_maps = []
    for c in range(NCORES):
        xc = inputs[c * BC : (c + 1) * BC]  # [64, 100, 700]
        xe = xc[:, 0::2, :]  # [64, 50, 700] (t = 0,2,..,98)
        xo = np.zeros((BC, K, C), dtype=np.float32)
        xo[:, 1:, :] = xc[:, 1:98:2, :]  # t = 1,3,..,97 -> k=1..49
        xe16 = xe.reshape(BC * K, C).T.astype(np.float16).copy()
        xo16 = xo.reshape(BC * K, C).T.astype(np.float16).copy()
        in_maps.append(
            {
                "xe16": xe16,
                "xo16": xo16,
                "w16": w16,
                "wa16": wa16,
                "coefs": coefs,
            }
        )

    run_kwargs = {}
    if os.environ.get("MEMBRANE_TRACE_DIR"):
        run_kwargs["tmpdir"] = os.environ["MEMBRANE_TRACE_DIR"]
    res = bass_utils.run_bass_kernel_spmd(
        nc, in_maps, core_ids=list(range(NCORES)), **run_kwargs
    )
    LAST_RESULT = res

    inv_a = (1.0 / alpha).reshape(1, 1, D)
    mo_sc = ((1.0 - beta) / alpha).reshape(1, 1, D)
    me_sc = ((1.0 - beta) / (alpha * beta)).reshape(1, 1, D)

    syn_full = np.empty((B, T, D), dtype=np.float32)
    mem_full = np.empty((B, T, D), dtype=np.float32)
    for c in range(NCORES):
        r = res.results[c]["out"].astype(np.float32)  # [512, NQ*OTW]
        for q in range(NQ):
            blk = r[:, q * OTW : (q + 1) * OTW]
            so = blk[:, 0:UO].reshape(D, 16, K)  # t=2k+1
            V = blk[:, UO : 2 * UO].reshape(D, 16, K)
            Sev = blk[:, 2 * UO : 3 * UO].reshape(D, 16, K)
            Mev = blk[:, 3 * UO : OTW].reshape(D, 16, K)
            b0 = c * BC + q * 16
            syn_full[b0 : b0 + 16, 1::2, :] = so.transpose(1, 2, 0)
            syn_full[b0 : b0 + 16, 0::2, :] = Sev.transpose(1, 2, 0) * inv_a
            mem_full[b0 : b0 + 16, 1::2, :] = V.transpose(1, 2, 0) * mo_sc
            mem_full[b0 : b0 + 16, 0::2, :] = Mev.transpose(1, 2, 0) * me_sc
    return (syn_full, mem_full)
